# revision 1
# baseline (speedup 1.0000x reference)
"""MultiHeadAttention Trainium2 kernel (8-core SPMD).

Problem: B=2, S=2048, DIM=1024, 16 heads, head_dim=64, fp32.
Sharding: core c -> (batch b = c//4, head-group g = c%4, 4 heads each).
Each core computes, for its batch and 4 heads:
    q = x Wq'^T            (Wq' = SCALE*Wq, no bias -- see bias algebra below)
    k = x Wk^T             (no bias)
    v = x Wv^T             (no bias)
    S^T[k,q] = k . q       (feature-major layout, row-packed 2 heads/matmul)
    P^T = exp(S^T) scaled per-k by m[k] = exp(SCALE * bq . k[k])
    outT[d,q] = sum_k V'[k,d] P^T[k,q]   with V' = diag(m) [V | 1]
    attn^T = outT[0:64] / outT[64]       (per-q softmax denominator)
    partial = attn^T . P_g^T             ([seq, 1024] output-projection partial)
Host sums the 4 per-group partials per batch and adds
bv @ proj_w.T + proj_b (V-bias and proj-bias commute through softmax/proj).

Bias algebra: softmax over k of SCALE*(q0+bq).(k0+bk) equals softmax of
(SCALE*q0).k0 + SCALE*bq.k0[k] -- the q0.bk and bq.bk terms are constant in k
and drop out. The per-k term is applied multiplicatively (m[k]) by scaling V
rows, and V's bias bv adds exactly bv to every attention output row.
"""

import numpy as np

import concourse.bass as bass
import concourse.mybir as mybir
import concourse.tile as tile
from concourse import bacc
from concourse import bass_utils

F32 = mybir.dt.float32
F32R = mybir.dt.float32r
BF16 = mybir.dt.bfloat16

P = 128
DIM = 1024
S = 2048
NH = 16
DH = 64
SCALE = 1.0 / 8.0
DC = DIM // P           # 8 contraction chunks
NST = S // 512          # 4 seq tiles of 512
NCH = S // P            # 16 kpos chunks of 128
FPC = 256               # features per core (4 heads * 64)
EG = 2                  # k-chunks per batched exp instruction
EB = 6                  # exp-tile bufs
SB = 3                  # staging bufs (un/out)


def _r(ap):
    return ap


def build_attention_bass():
    nc = bacc.Bacc(
        "TRN2",
        target_bir_lowering=False,
        debug=False,
        enable_asserts=False,
        num_devices=8,
    )
    xT = nc.dram_tensor("xT", [DIM, S], BF16, kind="ExternalInput").ap()
    wqT = nc.dram_tensor("wqT", [DIM, FPC], BF16, kind="ExternalInput").ap()
    wkT = nc.dram_tensor("wkT", [DIM, FPC], BF16, kind="ExternalInput").ap()
    wvT = nc.dram_tensor("wvT", [DIM, FPC], BF16, kind="ExternalInput").ap()
    bqz = nc.dram_tensor("bqz", [P, 2, 2], BF16, kind="ExternalInput").ap()
    pjT = nc.dram_tensor("pjT", [FPC, DIM], BF16, kind="ExternalInput").ap()
    out = nc.dram_tensor("out", [S, DIM], F32, kind="ExternalOutput").ap()

    with tile.TileContext(nc) as tc:
        _attention_body(tc, xT, wqT, wkT, wvT, bqz, pjT, out)
    nc.compile()
    return nc


def _attention_body(tc, xT, wqT, wkT, wvT, bqz, pjT, out):
    nc = tc.nc
    Exp = mybir.ActivationFunctionType.Exp
    Mult = mybir.AluOpType.mult

    with (
        tc.tile_pool(name="const", bufs=1) as cpool,
        tc.tile_pool(name="work", bufs=1) as wpool,
        tc.tile_pool(name="exp", bufs=EB) as epool,
        tc.tile_pool(name="stage", bufs=2) as spool,
        tc.tile_pool(name="ps", bufs=2, space="PSUM") as pspool,
        tc.tile_pool(name="psmm", bufs=2, space="PSUM") as pmmpool,
        tc.tile_pool(name="psav", bufs=2, space="PSUM") as pavpool,
    ):
        # ---- input loads (order = availability priority) -----------------
        wq_sb = cpool.tile([P, DC, FPC], BF16)
        wqT_r = wqT.rearrange("(dc p) f -> p dc f", p=P)
        nc.sync.dma_start(wq_sb[:, 0:2, :], wqT_r[:, 0:2, :])
        xt = cpool.tile([P, DC, S], BF16)
        xT_r = xT.rearrange("(dc p) s -> p dc s", p=P)
        nc.sync.dma_start(xt[:, 0:2, 0:512], xT_r[:, 0:2, 0:512])
        nc.sync.dma_start(wq_sb[:, 2:DC, :], wqT_r[:, 2:DC, :])
        nc.sync.dma_start(xt[:, 2:DC, 0:512], xT_r[:, 2:DC, 0:512])
        wk_sb = cpool.tile([P, DC, FPC], BF16)
        nc.sync.dma_start(wk_sb, wkT.rearrange("(dc p) f -> p dc f", p=P))
        for st in range(1, NST):
            sl = slice(512 * st, 512 * (st + 1))
            nc.sync.dma_start(xt[:, :, sl], xT_r[:, :, sl])
        wv_sb = cpool.tile([P, DC, FPC], BF16)
        nc.sync.dma_start(wv_sb, wvT.rearrange("(dc p) f -> p dc f", p=P))
        bq_sb = cpool.tile([P, 2, 2], BF16)
        nc.sync.dma_start(bq_sb, bqz)
        pj_sb = cpool.tile([P, 2, DIM], BF16)
        nc.sync.dma_start(pj_sb, pjT.rearrange("(c p) o -> p c o", p=P))

        q_sb = wpool.tile([P, 2, S], BF16)    # [dh-in-pair, pair, seq]
        k_sb = wpool.tile([P, 2, S], BF16)
        v_sb = wpool.tile([P, NCH, 4, DH + 1], BF16)
        m_sb = wpool.tile([P, NCH, 4], F32)   # exp(c) per (kpos, chunk, head)
        at_sb = wpool.tile([P, 2, S], BF16)   # normalized attn^T

        # ---- PE warm-up during the DMA lead-in ---------------------------
        warm = wpool.tile([P, 512], BF16)
        nc.vector.memset(warm, 1.0)
        wps = pmmpool.tile([P, 512], F32, tag="mm")
        for i in range(28):
            nc.tensor.matmul(wps, lhsT=warm[:, 0:P], rhs=warm,
                             start=True, stop=True)

        def qk_tile(p, wsb, dest, st):
            ps = pmmpool.tile([P, 512], F32, tag="mm")
            for dc in range(DC):
                nc.tensor.matmul(
                    ps,
                    lhsT=wsb[:, dc, P * p:P * (p + 1)],
                    rhs=xt[:, dc, 512 * st:512 * (st + 1)],
                    start=(dc == 0),
                    stop=(dc == DC - 1),
                )
            nc.vector.tensor_copy(dest[:, p, 512 * st:512 * (st + 1)], ps)

        def c_and_m(p):
            # c[k] = SCALE * bq_h . k0_h[k] via block-diagonal bq operand.
            c_ps = pmmpool.tile([P, 512], F32, tag="mm")
            for ch in range(NCH):
                nc.tensor.matmul(
                    c_ps[:, 2 * ch:2 * ch + 2],
                    lhsT=k_sb[:, p, P * ch:P * (ch + 1)],
                    rhs=bq_sb[:, p, :],
                    start=True,
                    stop=True,
                )
            for h in (0, 1):
                hh = 2 * p + h
                nc.scalar.activation(
                    m_sb[:, :, hh],
                    c_ps[:, 0:2 * NCH].rearrange("p (ch h) -> p ch h", h=2)[:, :, h],
                    Exp,
                )
                # denominator column of V' is exp(c) itself
                nc.vector.tensor_copy(v_sb[:, :, hh, DH], m_sb[:, :, hh])

        def v_chunk(ch):
            ps = pmmpool.tile([P, 512], F32, tag="mm")
            for dc in range(DC):
                nc.tensor.matmul(
                    ps[:, 0:FPC],
                    lhsT=xt[:, dc, P * ch:P * (ch + 1)],
                    rhs=wv_sb[:, dc, :],
                    start=(dc == 0),
                    stop=(dc == DC - 1),
                )
            nc.vector.tensor_copy(
                v_sb[:, ch, :, 0:DH],
                ps[:, 0:FPC].rearrange("p (h d) -> p h d", h=4),
            )
            scale_v(0, ch)

        def scale_v(p, ch):
            nc.vector.tensor_tensor(
                v_sb[:, ch, 2 * p:2 * p + 2, 0:DH],
                v_sb[:, ch, 2 * p:2 * p + 2, 0:DH],
                m_sb[:, ch, 2 * p:2 * p + 2, None].to_broadcast([P, 2, DH]),
                Mult,
            )

        def proj_tile(sm, nt):
            ps = pmmpool.tile([P, 512], F32, tag="mm")
            for pc in range(2):
                nc.tensor.matmul(
                    ps,
                    lhsT=at_sb[:, pc, P * sm:P * (sm + 1)],
                    rhs=pj_sb[:, pc, 512 * nt:512 * (nt + 1)],
                    start=(pc == 0),
                    stop=(pc == 1),
                )
            stg = spool.tile([P, 512], F32, tag="out", bufs=SB)
            nc.vector.tensor_copy(stg, ps)
            nc.sync.dma_start(
                out[P * sm:P * (sm + 1), 512 * nt:512 * (nt + 1)], stg
            )

        def attention_unit(p, qt, fillers):
            """One (pair, qtile) unit: both heads interleaved for
            score row-group concurrency. fillers: list of thunks to
            emit inside the ACT-paced window (one per group)."""
            qsl = slice(512 * qt, 512 * (qt + 1))
            pav = [pavpool.tile([P, 512], F32, tag="av", name=f"pav_{p}_{qt}_{h}")
                   for h in (0, 1)]
            for g in range(NCH // EG):
                if fillers:
                    fillers.pop(0)()
                st_t = [pspool.tile([P, EG, 512], F32, tag="st", name=f"st_{p}_{qt}_{g}_{h}")
                        for h in (0, 1)]
                for j in range(EG):
                    ch = EG * g + j
                    for h in (0, 1):
                        nc.tensor.matmul(
                            st_t[h][:, j, :],
                            lhsT=k_sb[DH * h:DH * (h + 1), p, P * ch:P * (ch + 1)],
                            rhs=q_sb[DH * h:DH * (h + 1), p, qsl],
                            start=True,
                            stop=True,
                        )
                e_t = [epool.tile([P, EG, 512], BF16, tag="e", name=f"e_{p}_{qt}_{g}_{h}")
                       for h in (0, 1)]
                for h in (0, 1):
                    nc.scalar.activation(e_t[h], st_t[h], Exp)
                for j in range(EG):
                    ch = EG * g + j
                    for h in (0, 1):
                        nc.tensor.matmul(
                            pav[h][0:DH + 1, :],
                            lhsT=v_sb[:, ch, 2 * p + h, :],
                            rhs=e_t[h][:, j, :],
                            start=(ch == 0),
                            stop=(ch == NCH - 1),
                        )
            while fillers:
                fillers.pop(0)()
            un = [spool.tile([DH + 1, 512], F32, tag="un", bufs=SB, name=f"un_{p}_{qt}_{h}")
                  for h in (0, 1)]
            for h in (0, 1):
                nc.vector.tensor_copy(un[h], pav[h][0:DH + 1, :])
            for h in (0, 1):
                rec = spool.tile([1, 512], F32, tag="rec")
                nc.vector.reciprocal(rec, un[h][DH:DH + 1, :])
                rb = spool.tile([DH, 512], F32, tag="rb")
                nc.gpsimd.partition_broadcast(rb, rec)
                nc.vector.tensor_tensor(
                    at_sb[DH * h:DH * (h + 1), p, qsl],
                    un[h][0:DH, :],
                    rb,
                    Mult,
                )

        # ---- emission: attention windows absorb the side work ------------
        for st in range(NST):
            qk_tile(0, wq_sb, q_sb, st)
        for st in range(NST):
            qk_tile(0, wk_sb, k_sb, st)
        c_and_m(0)
        for ch in range(6):
            v_chunk(ch)

        # attn0/qt0: remaining V chunks just-in-time inside the unit
        attention_unit(0, 0, [
            (lambda c0=c: (v_chunk(2 * c0 + 6), v_chunk(2 * c0 + 7)))
            for c in range(5)
        ])
        # attn0/qt1-2: pair-1 q/k projections as filler
        attention_unit(0, 1, [
            (lambda s=st: qk_tile(1, wq_sb, q_sb, s)) for st in range(NST)
        ])
        attention_unit(0, 2, [
            (lambda s=st: qk_tile(1, wk_sb, k_sb, s)) for st in range(NST)
        ])
        # attn0/qt3: pair-1 c/m + V rescale as filler
        attention_unit(0, 3, [lambda: c_and_m(1)] + [
            (lambda c0=c: (scale_v(1, 2 * c0), scale_v(1, 2 * c0 + 1)))
            for c in range(NCH // 2)
        ])

        # attn1: proj for finished qtiles as filler
        attention_unit(1, 0, [])
        for qt in range(1, NST):
            prev = qt - 1
            attention_unit(1, qt, [
                (lambda s=sm, n=nt: proj_tile(s, n))
                for sm in range(4 * prev, 4 * prev + 4) for nt in range(2)
            ])
        for sm in range(12, 16):
            for nt in range(2):
                proj_tile(sm, nt)


# ----------------------------------------------------------------------------
# host-side wrapper
# ----------------------------------------------------------------------------

_NC_CACHE = {}


def _get_nc():
    if "nc" not in _NC_CACHE:
        _NC_CACHE["nc"] = build_attention_bass()
    return _NC_CACHE["nc"]


def make_in_maps(x, qkv_w, qkv_b, proj_w):
    """Build the 8 per-core input dicts (host-side sharding)."""
    import ml_dtypes

    bf16 = ml_dtypes.bfloat16
    in_maps = []
    for c in range(8):
        b, g = divmod(c, 4)
        fsl = slice(g * FPC, (g + 1) * FPC)
        wq = (SCALE * qkv_w[0 * DIM:1 * DIM][fsl]).T     # (1024, 256)
        wk = qkv_w[1 * DIM:2 * DIM][fsl].T
        wv = qkv_w[2 * DIM:3 * DIM][fsl].T
        bq = SCALE * qkv_b[0 * DIM:1 * DIM][fsl]         # (256,)
        bqz = np.zeros((P, 2, 2), np.float32)
        for p in range(2):
            for h in range(2):
                bqz[DH * h:DH * (h + 1), p, h] = bq[(2 * p + h) * DH:(2 * p + h + 1) * DH]
        pj = proj_w[:, fsl].T                            # (256, 1024)
        in_maps.append({
            "xT": np.ascontiguousarray(x[b].T).astype(bf16),
            "wqT": np.ascontiguousarray(wq).astype(bf16),
            "wkT": np.ascontiguousarray(wk).astype(bf16),
            "wvT": np.ascontiguousarray(wv).astype(bf16),
            "bqz": bqz.astype(bf16),
            "pjT": np.ascontiguousarray(pj).astype(bf16),
        })
    return in_maps


def combine_outputs(results, qkv_b, proj_w, proj_b):
    """Sum per-group partials and add the host-folded biases."""
    bv = qkv_b[2 * DIM:3 * DIM]
    host_bias = bv @ proj_w.T + proj_b                   # (1024,)
    out = np.empty((2, S, DIM), np.float32)
    for b in range(2):
        acc = np.zeros((S, DIM), np.float32)
        for g in range(4):
            acc += results[4 * b + g]["out"]
        out[b] = acc + host_bias[None, :]
    return out


def kernel(x, qkv_w, qkv_b, proj_w, proj_b):
    x = np.asarray(x, np.float32)
    qkv_w = np.asarray(qkv_w, np.float32)
    qkv_b = np.asarray(qkv_b, np.float32)
    proj_w = np.asarray(proj_w, np.float32)
    proj_b = np.asarray(proj_b, np.float32)

    nc = _get_nc()
    in_maps = make_in_maps(x, qkv_w, qkv_b, proj_w)
    res = bass_utils.run_bass_kernel_spmd(nc, in_maps, core_ids=list(range(8)))
    return combine_outputs(res.results, qkv_b, proj_w, proj_b)

